# revision 1
# baseline (speedup 1.0000x reference)
"""Trainium2 Bass kernel for nn_MetaHeteroLinear (moe_routing).

out[n] = x[n] @ W[type_vec[n]] + B[type_vec[n]],
with W [8,128,128] / B [8,128] generated from edge_feas by two small MLPs.

Strategy (8 NeuronCores, data parallel over rows; 62500 rows/core):
 - Host computes routing tables only (argsort by type + padding); all data
   movement and math runs on device.
 - Each core's rows split into 2 sub-shards (31232 / 31268 rows) so every
   dma_gather source/staging index fits int16 (<32768).
 - Pass 1 (per sub-shard): dma_gather x rows in sorted-by-type order (32
   tiles of 128 rows per type, fixed capacity -> per-tile weight is static),
   PE-transpose each tile, fp32 matmul against resident per-type weights,
   bias add, dense-write results to a DRAM staging buffer.
 - Pass 2: dma_gather from staging with the inverse permutation, dense-write
   the output rows in natural order.
 - Generator MLPs computed on every core (replicated weights).
"""
import numpy as np

import concourse.bass as bass
import concourse.bacc as bacc
import concourse.tile as tile
import concourse.mybir as mybir
from concourse.bass_utils import run_bass_kernel_spmd
from concourse.masks import make_identity

P = 128
IN_C = 128
OUT_C = 128
MEM = 512
HID = 256
T = 8
IO = IN_C * OUT_C  # 16384

N_CORES = 8
N = 500_000
R = N // N_CORES           # 62500 rows per core
SUB_BOUND = 244 * P        # 31232: sub-shard A = [0, SUB_BOUND), B = rest
TPT = 32                   # tiles (of 128 rows) per type per sub-shard
SLOTS = T * TPT * P        # 32768 staging slots per sub-shard
CAP = TPT * P              # 4096 rows per type capacity
# pass-1 gather calls: one per (sub, type) = 16 calls of 4096 rows
# pass-2 calls: sub A 244 tiles -> 7x32 + 20; sub B 245 tiles -> 7x32 + 21
P2_CALLS = [(0, k) for k in [32] * 7 + [20]] + [(1, k) for k in [32] * 7 + [21]]
OUT_ROWS = 489 * P         # 62592 (tail 92 rows ignored by host)

f32 = mybir.dt.float32
i16 = mybir.dt.int16
RELU = mybir.ActivationFunctionType.Relu

_CACHE = {}
LAST_RESULTS = None  # BassKernelResults of the most recent run (for test harness)

WEIGHT_NAMES = [
    "edge_feas", "wg_w1", "wg_b1", "wg_w2", "wg_b2", "wg_w3", "wg_b3",
    "bg_w1", "bg_b1", "bg_w2", "bg_b2", "bg_w3", "bg_b3",
]

G1_COLS = CAP // 16                     # 256 cols per pass-1 call
G2_COLS = [k * P // 16 for _, k in P2_CALLS]
G2_OFF = np.concatenate([[0], np.cumsum(G2_COLS)]).astype(int)
G2_TOT = int(G2_OFF[-1])                # 3912


def _gen_hidden(nc, cpool, pspool, edgeT_sb, w1_ap, b1_ap, w2_ap, b2_ap, tagp):
    """Two MLP hidden layers, transposed: edgeT [128,4,8] -> h2T [128,2,8]."""
    w1_sb = cpool.tile([P, 4, HID], f32, tag=tagp + "w1")
    nc.sync.dma_start(out=w1_sb[:], in_=w1_ap.rearrange("(c p) h -> p c h", p=P))
    b1T = cpool.tile([P, 2], f32, tag=tagp + "b1")
    nc.sync.dma_start(out=b1T[:], in_=b1_ap.rearrange("(c p) -> p c", p=P))
    w2_sb = cpool.tile([P, 2, HID], f32, tag=tagp + "w2")
    nc.sync.dma_start(out=w2_sb[:], in_=w2_ap.rearrange("(c p) h -> p c h", p=P))
    b2T = cpool.tile([P, 2], f32, tag=tagp + "b2")
    nc.sync.dma_start(out=b2T[:], in_=b2_ap.rearrange("(c p) -> p c", p=P))

    h1T = cpool.tile([P, 2, T], f32, tag=tagp + "h1")
    for m in range(2):
        ps = pspool.tile([P, T], f32, tag="gen_ps")
        for kc in range(4):
            nc.tensor.matmul(ps[:], lhsT=w1_sb[:, kc, m * P:(m + 1) * P],
                             rhs=edgeT_sb[:, kc, :], start=(kc == 0), stop=(kc == 3))
        nc.scalar.activation(h1T[:, m, :], ps[:], RELU, bias=b1T[:, m:m + 1])
    h2T = cpool.tile([P, 2, T], f32, tag=tagp + "h2")
    for m in range(2):
        ps = pspool.tile([P, T], f32, tag="gen_ps")
        for kc in range(2):
            nc.tensor.matmul(ps[:], lhsT=w2_sb[:, kc, m * P:(m + 1) * P],
                             rhs=h1T[:, kc, :], start=(kc == 0), stop=(kc == 1))
        nc.scalar.activation(h2T[:, m, :], ps[:], RELU, bias=b2T[:, m:m + 1])
    return h2T


def _build_generators(nc, tc, ident, ones_sb, d, wcat_sb, bbc_sb, wtmp_d, btmp_d):
    with tc.tile_pool(name="gen", bufs=1) as gpool, \
         tc.tile_pool(name="gen2", bufs=2) as g2pool, \
         tc.tile_pool(name="genps", bufs=2, space="PSUM") as genps:
        # edge transpose: [8, 512] -> edgeT [128, 4, 8]
        edge_sb = gpool.tile([T, MEM], f32)
        nc.sync.dma_start(out=edge_sb[:], in_=d["edge_feas"][:])
        edgeT = gpool.tile([P, 4, T], f32)
        for kc in range(4):
            ps = genps.tile([P, T], f32, tag="gen_ps")
            nc.tensor.transpose(ps[:], edge_sb[:, kc * P:(kc + 1) * P], ident[:T, :T])
            nc.vector.tensor_copy(edgeT[:, kc, :], ps[:])

        # ---- W generator ----
        h2T = _gen_hidden(nc, gpool, genps, edgeT, d["wg_w1"], d["wg_b1"],
                          d["wg_w2"], d["wg_b2"], "wg")
        for n in range(IO // 512):
            w3_sb = g2pool.tile([P, 2, 512], f32, tag="w3")
            nc.sync.dma_start(
                out=w3_sb[:],
                in_=d["wg_w3"].rearrange("(c p) n -> p c n", p=P)
                [:, :, n * 512:(n + 1) * 512])
            ps = genps.tile([T, 512], f32, tag="w_ps")
            for kc in range(2):
                nc.tensor.matmul(ps[:], lhsT=h2T[:, kc, :], rhs=w3_sb[:, kc, :],
                                 start=(kc == 0), stop=(kc == 1))
            wf_sb = g2pool.tile([T, 512], f32, tag="wf")
            nc.vector.tensor_copy(wf_sb[:], ps[:])
            nc.sync.dma_start(out=wtmp_d[:, n * 512:(n + 1) * 512], in_=wf_sb[:])
        # DRAM round-trip rearrange [t,(i,o)] -> [i,t,o]
        nc.sync.dma_start(out=wcat_sb[:],
                          in_=wtmp_d.rearrange("t (i o) -> i t o", i=P))
        b3T = gpool.tile([P, OUT_C], f32)
        nc.sync.dma_start(out=b3T[:], in_=d["wg_b3"].rearrange("(i o) -> i o", i=P))
        for t in range(T):
            nc.vector.tensor_add(wcat_sb[:, t, :], wcat_sb[:, t, :], b3T[:])

        # ---- B generator ----
        h2bT = _gen_hidden(nc, gpool, genps, edgeT, d["bg_w1"], d["bg_b1"],
                           d["bg_w2"], d["bg_b2"], "bg")
        bw3_sb = gpool.tile([P, 2, OUT_C], f32)
        nc.sync.dma_start(out=bw3_sb[:],
                          in_=d["bg_w3"].rearrange("(c p) h -> p c h", p=P))
        bb3_sb = gpool.tile([1, OUT_C], f32)
        nc.sync.dma_start(out=bb3_sb[:], in_=d["bg_b3"][None, :])
        ps_b = genps.tile([T, OUT_C], f32, tag="w_ps")
        for kc in range(2):
            nc.tensor.matmul(ps_b[:], lhsT=h2bT[:, kc, :], rhs=bw3_sb[:, kc, :],
                             start=(kc == 0), stop=False)
        nc.tensor.matmul(ps_b[:], lhsT=ones_sb[:1, :T], rhs=bb3_sb[:1, :],
                         start=False, stop=True)
        b_sb = gpool.tile([T, OUT_C], f32)
        nc.vector.tensor_copy(b_sb[:], ps_b[:])
        nc.sync.dma_start(out=btmp_d.rearrange("(t o) -> t o", t=T), in_=b_sb[:])
        brow = gpool.tile([1, T * OUT_C], f32)
        nc.sync.dma_start(out=brow[:], in_=btmp_d[None, :])
        for c in range(2):
            bb_ps = genps.tile([P, 512], f32, tag="bb_ps")
            nc.tensor.matmul(bb_ps[:], lhsT=ones_sb[:1, :P],
                             rhs=brow[:1, c * 512:(c + 1) * 512], start=True, stop=True)
            nc.vector.tensor_copy(
                bbc_sb[:].rearrange("p t o -> p (t o)")[:, c * 512:(c + 1) * 512],
                bb_ps[:])


def _build_nc():
    nc = bacc.Bacc("TRN2", target_bir_lowering=False, debug=False)
    d = {}
    x_d = nc.dram_tensor("x_s", [R, IN_C], f32, kind="ExternalInput")
    g1_d = nc.dram_tensor("g1idx", [P, 16 * G1_COLS], i16, kind="ExternalInput")
    g2_d = nc.dram_tensor("g2idx", [P, G2_TOT], i16, kind="ExternalInput")
    shapes = {
        "edge_feas": [T, MEM],
        "wg_w1": [MEM, HID], "wg_b1": [HID], "wg_w2": [HID, HID], "wg_b2": [HID],
        "wg_w3": [HID, IO], "wg_b3": [IO],
        "bg_w1": [MEM, HID], "bg_b1": [HID], "bg_w2": [HID, HID], "bg_b2": [HID],
        "bg_w3": [HID, OUT_C], "bg_b3": [OUT_C],
    }
    for name, shp in shapes.items():
        d[name] = nc.dram_tensor(name, shp, f32, kind="ExternalInput")
    out_d = nc.dram_tensor("out_s", [OUT_ROWS, OUT_C], f32, kind="ExternalOutput")
    wtmp_d = nc.dram_tensor("wtmp", [T, IO], f32)
    btmp_d = nc.dram_tensor("btmp", [T * OUT_C], f32)
    stg_d = [nc.dram_tensor(f"stg{s}", [SLOTS, OUT_C], f32) for s in range(2)]

    with tile.TileContext(nc) as tc:
        with tc.tile_pool(name="const", bufs=1) as cpool, \
             tc.tile_pool(name="io", bufs=3) as iopool, \
             tc.tile_pool(name="work", bufs=4) as wpool:

            ident = cpool.tile([P, P], f32)
            make_identity(nc, ident[:])
            ones_sb = cpool.tile([1, P], f32)
            nc.vector.memset(ones_sb[:], 1.0)
            g1_sb = cpool.tile([P, 16 * G1_COLS], i16)
            nc.sync.dma_start(out=g1_sb[:], in_=g1_d[:])
            g2_sb = cpool.tile([P, G2_TOT], i16)
            nc.sync.dma_start(out=g2_sb[:], in_=g2_d[:])

            wcat_sb = cpool.tile([P, T, OUT_C], f32)   # [in_c, t, out_c]
            bbc_sb = cpool.tile([P, T, OUT_C], f32)    # B[t] broadcast over partitions

            _build_generators(nc, tc, ident, ones_sb, d, wcat_sb, bbc_sb,
                              wtmp_d, btmp_d)

            with tc.tile_pool(name="ps", bufs=3, space="PSUM") as pspool:
                # ---------------- pass 1: gather-sorted compute ----------------
                for call in range(16):
                    sub, t = divmod(call, T)
                    lo = 0 if sub == 0 else SUB_BOUND
                    hi = SUB_BOUND if sub == 0 else R
                    xg = iopool.tile([P, TPT, IN_C], f32, tag="xg")
                    nc.gpsimd.dma_gather(
                        out_ap=xg[:],
                        in_ap=x_d[lo:hi, :],
                        idxs_ap=g1_sb[:, call * G1_COLS:(call + 1) * G1_COLS],
                        num_idxs=CAP, num_idxs_reg=CAP, elem_size=IN_C,
                        single_packet=False)
                    y_sb = iopool.tile([P, TPT, OUT_C], f32, tag="y")
                    for j in range(TPT):
                        xT_ps = pspool.tile([P, P], f32, tag="xT")
                        nc.tensor.transpose(xT_ps[:], xg[:, j, :], ident[:])
                        xT_sb = wpool.tile([P, P], f32, tag="xTs")
                        nc.scalar.copy(xT_sb[:], xT_ps[:])
                        y_ps = pspool.tile([P, P], f32, tag="y")
                        nc.tensor.matmul(y_ps[:], lhsT=xT_sb[:], rhs=wcat_sb[:, t, :],
                                         start=True, stop=True)
                        nc.vector.tensor_add(y_sb[:, j, :], y_ps[:], bbc_sb[:, t, :])
                    # dense staging write: within this call's 4096-slot block,
                    # staging row = p*TPT + j  <-  y_sb[p, j, :]
                    nc.sync.dma_start(
                        out=stg_d[sub][t * CAP:(t + 1) * CAP, :]
                        .rearrange("(p j) c -> p j c", p=P),
                        in_=y_sb[:])

                # -------------- pass 2: inverse gather, dense out --------------
                r0 = 0
                for ci, (sub, k) in enumerate(P2_CALLS):
                    ni = k * P
                    yg = iopool.tile([P, k, OUT_C], f32, tag="xg")
                    nc.gpsimd.dma_gather(
                        out_ap=yg[:],
                        in_ap=stg_d[sub][:],
                        idxs_ap=g2_sb[:, int(G2_OFF[ci]):int(G2_OFF[ci + 1])],
                        num_idxs=ni, num_idxs_reg=ni, elem_size=OUT_C,
                        single_packet=False)
                    nc.sync.dma_start(
                        out=out_d[r0:r0 + ni, :].rearrange("(p j) c -> p j c", p=P),
                        in_=yg[:])
                    r0 += ni
    nc.compile()
    return nc


def _wrap16(v):
    """flat int16 list -> [128, len/16] wrapped (idx i at [i%16, i//16]),
    replicated to all 8 Q7 core groups."""
    cols = len(v) // 16
    m = v.reshape(cols, 16).T
    return np.tile(m, (8, 1))


def _routing(tv_core):
    """tv_core: [R] types -> (g1 [128, 16*G1_COLS] i16, g2 [128, G2_TOT] i16,
    overflow core-local row ids)."""
    g1_parts = []
    g2val = np.zeros(R, np.int16)    # staging row (within own sub) per local row
    overflow = []
    for sub in range(2):
        lo = 0 if sub == 0 else SUB_BOUND
        hi = SUB_BOUND if sub == 0 else R
        tvs = tv_core[lo:hi]
        order = np.argsort(tvs, kind="stable").astype(np.int64)
        counts = np.bincount(tvs, minlength=T)
        sorted_rows = np.zeros(SLOTS, np.int64)   # slot -> sub-local row (pad->0)
        srt_pos = np.full(hi - lo, -1, np.int64)  # sub-local row -> slot
        start = 0
        for t in range(T):
            cnt = int(counts[t])
            seg = order[start:start + cnt]
            start += cnt
            if cnt > CAP:
                overflow.extend((seg[CAP:] + lo).tolist())
                seg = seg[:CAP]
                cnt = CAP
            base = t * CAP
            sorted_rows[base:base + cnt] = seg
            srt_pos[seg] = base + np.arange(cnt)
        g1_parts.append(sorted_rows.astype(np.int16))
        # staging row for slot s: (s//4096)*4096 + (s%128)*32 + (s//128)%32
        s = srt_pos
        stg_row = (s // CAP) * CAP + (s % P) * TPT + (s // P) % TPT
        ok = s >= 0
        g2val[lo:hi][ok] = stg_row[ok].astype(np.int16)
    g1 = _wrap16(np.concatenate(g1_parts))

    g2_list = []
    r0 = 0
    for sub, k in P2_CALLS:
        ni = k * P
        i = np.arange(ni)
        n = r0 + (i % P) * k + (i // P)        # natural core-local row per slot
        vals = np.where(n < R, g2val[np.minimum(n, R - 1)], 0).astype(np.int16)
        g2_list.append(vals)
        r0 += ni
    g2 = _wrap16(np.concatenate(g2_list))
    return np.ascontiguousarray(g1), np.ascontiguousarray(g2), overflow


def _host_mlp(m, w1, b1, w2, b2, w3, b3):
    h = np.maximum(m @ w1 + b1, 0)
    h = np.maximum(h @ w2 + b2, 0)
    return h @ w3 + b3


def kernel(**inputs):
    global LAST_RESULTS
    x = np.ascontiguousarray(np.asarray(inputs["x"], dtype=np.float32))
    tv = np.asarray(inputs["type_vec"]).astype(np.int64)
    assert x.shape == (N, IN_C), x.shape
    weights = {k: np.ascontiguousarray(np.asarray(inputs[k], dtype=np.float32))
               for k in WEIGHT_NAMES}

    if "nc" not in _CACHE:
        _CACHE["nc"] = _build_nc()
    nc = _CACHE["nc"]

    in_maps = []
    overflows = []
    for c in range(N_CORES):
        sl = slice(c * R, (c + 1) * R)
        g1, g2, ovf = _routing(tv[sl])
        overflows.append(ovf)
        m = {"x_s": x[sl], "g1idx": g1, "g2idx": g2}
        m.update(weights)
        in_maps.append(m)

    res = run_bass_kernel_spmd(nc, in_maps, core_ids=list(range(N_CORES)))
    LAST_RESULTS = res

    out = np.empty((N, OUT_C), dtype=np.float32)
    for c in range(N_CORES):
        out[c * R:(c + 1) * R] = res.results[c]["out_s"][:R]

    # host fallback for (rare) per-type capacity overflow
    if any(overflows):
        w = weights
        W = _host_mlp(w["edge_feas"], w["wg_w1"], w["wg_b1"], w["wg_w2"], w["wg_b2"],
                      w["wg_w3"], w["wg_b3"]).reshape(T, IN_C, OUT_C)
        B = _host_mlp(w["edge_feas"], w["bg_w1"], w["bg_b1"], w["bg_w2"], w["bg_b2"],
                      w["bg_w3"], w["bg_b3"])
        for c in range(N_CORES):
            for r in overflows[c]:
                g = c * R + r
                t = int(tv[g])
                out[g] = x[g] @ W[t] + B[t]
    return out



# revision 7
# speedup vs baseline: 3.8829x; 3.8829x over previous
"""Trainium2 Bass kernel for nn_MetaHeteroLinear (moe_routing).

out[n] = x[n] @ W[type_vec[n]] + B[type_vec[n]],
with W [8,128,128] / B [8,128] generated from edge_feas by two small MLPs.

Strategy (8 NeuronCores, data parallel over rows; 62500 rows/core):
 - The generator MLPs are tiny (~70 MFLOP total); computed once on host in
   f32 and the resulting per-type W/B replicated to every core (per the
   sharding hint) — this avoids shipping the 16 MB wg_w3 weight 8x per call.
 - Host computes routing tables (argsort by type per half-shard so gather
   indices fit int16) and per-call valid counts.
 - Device, per (half-shard, type) call: transposed dma_gather pulls the
   rows of that type as x^T columns (bf16), 33 matmul tiles of 128 rows
   against the resident W[t] with the bias folded in via a 1-row seed
   matmul into PSUM, then dma_scatter_add writes each row's result
   directly to its natural output position (the output buffer is donated
   zero-filled, so += on untouched rows == store). Padding tokens are -1
   (skipped by both gather and scatter); per-call valid counts are loaded
   into gpsimd registers at runtime.
 - Everything moves as bf16 (rel-err ~3e-3, well under the 2e-2 gate),
   halving both tunnel directions vs f32.
 - The jit-wrapped NEFF executable is cached across calls; output zeros
   are produced on-device (never shipped); output fetch is threaded.
"""
import numpy as np
import ml_dtypes

import jax
import jax.numpy as jnp
from jax.experimental.shard_map import shard_map
from jax.sharding import Mesh, PartitionSpec, NamedSharding

import concourse.bass as bass
import concourse.bacc as bacc
import concourse.tile as tile
import concourse.mybir as mybir
from concourse import bass2jax

P = 128
IN_C = 128
OUT_C = 128
MEM = 512
HID = 256
T = 8

N_CORES = 8
N = 500_000
R = N // N_CORES           # 62500 rows per core
SUB = R // 2               # 31250: half-shards so gather idx fits int16
TPT = 33                   # tiles (of 128 rows) per (half, type) call
CAP = TPT * P              # 4224 row capacity per call (mean 3906 + 5.4 sigma)
NCALLS = 2 * T             # 16 calls per core
COLS = CAP // 16           # 264 idx columns per call

f32 = mybir.dt.float32
bf16 = mybir.dt.bfloat16
i16 = mybir.dt.int16
i32 = mybir.dt.int32
BF16 = ml_dtypes.bfloat16

_CACHE = {}
LAST_RESULTS = None  # kept for test harness compat (no NTFF profile available)


def _build_nc():
    nc = bacc.Bacc("TRN2", target_bir_lowering=False, debug=False)
    x_d = nc.dram_tensor("x_s", [R, IN_C], bf16, kind="ExternalInput")
    g1_d = nc.dram_tensor("g1idx", [P, NCALLS * COLS], i16, kind="ExternalInput")
    cnt_d = nc.dram_tensor("cnt", [1, NCALLS], i32, kind="ExternalInput")
    w_d = nc.dram_tensor("wt", [IN_C, T, OUT_C], bf16, kind="ExternalInput")
    b_d = nc.dram_tensor("bt", [1, T * OUT_C], bf16, kind="ExternalInput")
    out_d = nc.dram_tensor("out_s", [R, OUT_C], bf16, kind="ExternalOutput")

    with tile.TileContext(nc) as tc:
        with tc.tile_pool(name="const", bufs=1) as cpool, \
             tc.tile_pool(name="io", bufs=3) as iopool, \
             tc.tile_pool(name="ps", bufs=4, space="PSUM") as pspool:
            g1_sb = cpool.tile([P, NCALLS * COLS], i16)
            nc.sync.dma_start(out=g1_sb[:], in_=g1_d[:])
            cnt_sb = cpool.tile([1, NCALLS], i32)
            nc.sync.dma_start(out=cnt_sb[:], in_=cnt_d[:])
            wcat_sb = cpool.tile([P, T, OUT_C], bf16)   # [in_c, t, out_c]
            nc.sync.dma_start(out=wcat_sb[:], in_=w_d[:])
            bt_sb = cpool.tile([1, T * OUT_C], bf16)  # all biases on partition 0
            nc.sync.dma_start(out=bt_sb[:], in_=b_d[:])
            ones_sb = cpool.tile([1, P], bf16)
            nc.vector.memset(ones_sb[:], 1.0)

            regs = [nc.gpsimd.alloc_register(f"cnt{k}") for k in range(NCALLS)]
            pend = None  # (y_sb, idx slice, reg, out AP) awaiting scatter
            for call in range(NCALLS):
                sub, t = divmod(call, T)
                lo = sub * SUB
                hi = R if sub == 1 else SUB
                r = regs[call]
                nc.gpsimd.reg_load(r, cnt_sb[:1, call:call + 1])
                xT = iopool.tile([P, 1, CAP], bf16, tag="xT")
                idx_ap = g1_sb[:, call * COLS:(call + 1) * COLS]
                nc.gpsimd.dma_gather(
                    out_ap=xT[:], in_ap=x_d[lo:hi, :], idxs_ap=idx_ap,
                    num_idxs=CAP, num_idxs_reg=r, elem_size=IN_C,
                    transpose=True, single_packet=False)
                y_sb = iopool.tile([P, TPT, OUT_C], bf16, tag="y")
                for j in range(TPT):
                    ps = pspool.tile([P, OUT_C], f32, tag="ps")
                    nc.tensor.matmul(ps[:], lhsT=ones_sb[:1, :],
                                     rhs=bt_sb[:1, t * OUT_C:(t + 1) * OUT_C],
                                     start=True, stop=False)
                    nc.tensor.matmul(ps[:], lhsT=xT[:, 0, j * P:(j + 1) * P],
                                     rhs=wcat_sb[:, t, :], start=False, stop=True)
                    nc.scalar.copy(y_sb[:, j, :], ps[:])
                # issue the previous call's scatter after this call's gather so
                # the gather DMA overlaps the previous call's matmul tail
                if pend is not None:
                    nc.gpsimd.dma_scatter_add(
                        out_ap=pend[3], in_ap=pend[0][:], idxs_ap=pend[1],
                        num_idxs=CAP, num_idxs_reg=pend[2], elem_size=OUT_C,
                        single_packet=False)
                pend = (y_sb, idx_ap, r, out_d[lo:hi, :])
            nc.gpsimd.dma_scatter_add(
                out_ap=pend[3], in_ap=pend[0][:], idxs_ap=pend[1],
                num_idxs=CAP, num_idxs_reg=pend[2], elem_size=OUT_C,
                single_packet=False)
    nc.compile()
    return nc


def _make_runner():
    """Compile once; return (sharded_jit, zeros_fn, in_names)."""
    bass2jax.install_neuronx_cc_hook()
    nc = _build_nc()
    assert nc.dbg_addr is None
    part_name = nc.partition_id_tensor.name if nc.partition_id_tensor else None
    in_names, out_names, out_avals = [], [], []
    for alloc in nc.m.functions[0].allocations:
        if not isinstance(alloc, mybir.MemoryLocationSet):
            continue
        name = alloc.memorylocations[0].name
        if alloc.kind == "ExternalInput":
            if name != part_name:
                in_names.append(name)
        elif alloc.kind == "ExternalOutput":
            out_names.append(name)
            out_avals.append(jax.core.ShapedArray(
                tuple(alloc.tensor_shape), mybir.dt.np(alloc.dtype)))
    n_params, n_outs = len(in_names), len(out_names)
    all_names = in_names + out_names
    if part_name is not None:
        all_names = all_names + [part_name]
    all_names = tuple(all_names)

    def _body(*args):
        operands = list(args)
        if part_name is not None:
            operands.append(bass2jax.partition_id_tensor())
        return tuple(bass2jax._bass_exec_p.bind(
            *operands, out_avals=tuple(out_avals), in_names=all_names,
            out_names=tuple(out_names), lowering_input_output_aliases=(),
            sim_require_finite=True, sim_require_nnan=True, nc=nc))

    mesh = Mesh(np.asarray(jax.devices()[:N_CORES]), ("core",))
    spec = PartitionSpec("core")
    sharded = jax.jit(
        shard_map(_body, mesh=mesh, in_specs=(spec,) * (n_params + n_outs),
                  out_specs=(spec,) * n_outs, check_rep=False),
        donate_argnums=tuple(range(n_params, n_params + n_outs)),
        keep_unused=True)
    zeros_fn = jax.jit(lambda: jnp.zeros((N_CORES * R, OUT_C), jnp.bfloat16),
                       out_shardings=NamedSharding(mesh, spec))
    return sharded, zeros_fn, in_names


def _routing(tv_core):
    """tv_core: [R] int types -> (g1 [NCALLS, CAP] i16 with -1 pads,
    cnt [NCALLS] i32, overflow core-local row ids needing host fixup)."""
    g1 = np.full((NCALLS, CAP), -1, np.int16)
    cnt = np.zeros(NCALLS, np.int32)
    overflow = []
    for sub in range(2):
        lo, hi = sub * SUB, (R if sub == 1 else SUB)
        tvs = tv_core[lo:hi]
        order = np.argsort(tvs, kind="stable")
        counts = np.bincount(tvs, minlength=T)
        start = 0
        for t in range(T):
            c = int(counts[t])
            seg = order[start:start + c]
            start += c
            k = sub * T + t
            if c > CAP:
                overflow.extend((seg[CAP:] + lo).tolist())
                seg, c = seg[:CAP], CAP
            if c == 0:
                # hardware path needs >=1 valid token per call; sacrifice
                # local row 0 (scatter adds garbage there; host recomputes)
                g1[k, 0] = 0
                cnt[k] = 1
                overflow.append(lo)
            else:
                g1[k, :c] = seg.astype(np.int16)
                cnt[k] = c
    return g1, cnt, overflow


def _wrap16(flat):
    """flat int16 [NCALLS*CAP] -> [128, NCALLS*COLS] wrapped (token i at
    [i%16, i//16]), replicated to all 8 Q7 core groups."""
    m = flat.reshape(-1, 16).T
    return np.tile(m, (8, 1))


def _host_mlp(m, w1, b1, w2, b2, w3, b3):
    h = np.maximum(m @ w1 + b1, 0)
    h = np.maximum(h @ w2 + b2, 0)
    return h @ w3 + b3


def kernel(**inputs):
    x = np.ascontiguousarray(np.asarray(inputs["x"], dtype=np.float32))
    tv = np.asarray(inputs["type_vec"]).astype(np.int64)
    assert x.shape == (N, IN_C), x.shape
    ef = np.asarray(inputs["edge_feas"], dtype=np.float32)

    # per-type weights/biases from the tiny generator MLPs (host, f32)
    W = _host_mlp(ef, *[np.asarray(inputs[k], dtype=np.float32) for k in
                        ("wg_w1", "wg_b1", "wg_w2", "wg_b2", "wg_w3", "wg_b3")]
                  ).reshape(T, IN_C, OUT_C)
    B = _host_mlp(ef, *[np.asarray(inputs[k], dtype=np.float32) for k in
                        ("bg_w1", "bg_b1", "bg_w2", "bg_b2", "bg_w3", "bg_b3")])

    if "runner" not in _CACHE:
        _CACHE["runner"] = _make_runner()
    sharded, zeros_fn, in_names = _CACHE["runner"]

    zeros = zeros_fn()  # async on-device; overlaps with host prep below

    g1_g = np.empty((N_CORES * P, NCALLS * COLS), np.int16)
    cnt_g = np.empty((N_CORES, NCALLS), np.int32)
    overflows = []
    for c in range(N_CORES):
        g1, cnt, ovf = _routing(tv[c * R:(c + 1) * R])
        g1_g[c * P:(c + 1) * P] = _wrap16(g1.reshape(-1))
        cnt_g[c] = cnt
        overflows.append(ovf)

    w_g = np.broadcast_to(
        np.ascontiguousarray(W.transpose(1, 0, 2)).astype(BF16),
        (N_CORES, IN_C, T, OUT_C)).reshape(N_CORES * IN_C, T, OUT_C)
    b_g = np.broadcast_to(B.reshape(1, T * OUT_C).astype(BF16),
                          (N_CORES, T * OUT_C))

    glob = {"x_s": x.astype(BF16), "g1idx": g1_g, "cnt": cnt_g,
            "wt": np.ascontiguousarray(w_g), "bt": np.ascontiguousarray(b_g)}
    out_arr = sharded(*[glob[n] for n in in_names], zeros)[0]

    out = np.empty((N, OUT_C), dtype=np.float32)
    from concurrent.futures import ThreadPoolExecutor

    def fetch(s):
        lo = s.index[0].start or 0
        out[lo:lo + R] = np.asarray(s.data)  # bf16 -> f32 cast on assign

    with ThreadPoolExecutor(4) as ex:
        list(ex.map(fetch, out_arr.addressable_shards))

    if any(overflows):  # per-type capacity overflow: recompute those rows
        for c in range(N_CORES):
            for rr in set(overflows[c]):
                g = c * R + rr
                t = int(tv[g])
                out[g] = x[g] @ W[t] + B[t]
    return out
